# revision 1
# baseline (speedup 1.0000x reference)
"""Trainium2 Bass kernel for nn_EnergyDistributionCNN (3x3 conv -> unfold ->
softmax over patch -> weighted -> fold overlap-add), 8 NeuronCores.

Math (algebraically identical to the torch/jax reference):
    out = conv3x3(x, k)            cross-correlation, zero pad 1
    E   = exp(out)
    Z   = boxsum3x3(E padded with ONES)   (zero pads contribute exp(0)=1)
    U   = x / Z
    S   = boxsum3x3(U zero-padded)
    result = E * S

Sharding: row-block across 8 cores with a 3-row halo sliced on the host
(zero-filled at the global edges) -- no device-to-device communication.
Global boundary rows are handled uniformly by a per-row mask fused into the
exp's per-partition scale (exp(0*out)=1); boundary columns by host zero
padding plus static edge memsets.

On-core layout: rows on partitions, cols on the free dim, processed in
row-tiles (<=122 output rows) x width-halves. All vertical stencil mixing
runs on the TensorEngine via banded matrices; horizontal mixing is 3
column-shifted matmuls accumulated in PSUM. Everything on the PE uses
fp32r (full-rate moving operand, ~11-bit mantissa); the conv -- whose
error exp() amplifies -- is error-compensated with a hi/lo split:
    conv = Mhi @ Xhi + Mhi @ Xlo + Mlo @ Xhi       (~fp32 quality)
where Xhi is the fp32r-rounded x (DVE copy) and Xlo = x - Xhi.
exp runs on the ScalarEngine directly from conv's PSUM; 1/Z uses the DVE
fast reciprocal (~18 bits). Band row-mappings put every compute op at
partition base 0; the valid output rows sit at partitions [2, R+2), which
the (partition-unrestricted) output DMA reads.
"""

from contextlib import ExitStack

import numpy as np

import concourse.bacc as bacc
import concourse.mybir as mybir
import concourse.tile as tile
from concourse._compat import with_exitstack
from concourse.bass_utils import run_bass_kernel_spmd

F32 = mybir.dt.float32
F32R = mybir.dt.float32r

H = 4096
W = 4096
N_CORES = 8
RC = H // N_CORES  # rows per core
HALO = 3
RT = 122   # output rows per row-tile (RT + 6 <= 128 partitions)
WS = 2     # width splits (SBUF capacity)
WH = W // WS
C = 512    # matmul column chunk = one fp32 PSUM bank
NBUFS = 3
PS_BUFS = 3


# ---------------------------------------------------------------- host side

def _make_bands(k: np.ndarray) -> np.ndarray:
    """bands[v][p, m] = k[p-m, v] (conv, v=0..2); bands[3] = BB ones with
    p-m in 0..2 (S matmul); bands[4] = BT ones with m-p in 0..2 (Z).
    bands[5..9]: same five patterns as 4x block-diagonal 32x32 blocks, for
    the column-folded last row-tile."""
    bands = np.zeros((10, 128, 128), np.float32)
    idx = np.arange(128)
    for d in range(3):
        p = idx[d:]
        m = idx[: 128 - d]
        for v in range(3):
            bands[v, p, m] = k[d, v]
        bands[3, p, m] = 1.0
        bands[4, m, p] = 1.0
    for i in range(5):
        blk = bands[i][:32, :32]
        for b in range(4):
            bands[5 + i][32 * b : 32 * b + 32, 32 * b : 32 * b + 32] = blk
    return bands


def _make_core_inputs(x: np.ndarray, bands: np.ndarray, core: int):
    r0 = core * RC
    lo, hi = r0 - HALO, r0 + RC + HALO
    # 26 extra zero rows let the folded last tile load full 32-row blocks
    xh = np.zeros((RC + 2 * HALO + 26, W + 2 * HALO), np.float32)
    s_lo, s_hi = max(lo, 0), min(hi, H)
    xh[s_lo - lo : s_hi - lo, HALO : HALO + W] = x[s_lo:s_hi]
    gl = np.arange(lo, hi)
    mask = ((gl >= 0) & (gl < H)).astype(np.float32)[:, None]
    return {"xh": xh, "mask": mask, "bands": bands}


def _make_tiles():
    tiles = []
    o = 0
    while o < RC:
        R = min(RT, RC - o)
        tiles.append((o, R))
        o += R
    return tiles


def _chunks(total: int):
    out = []
    s = 0
    while s < total:
        out.append((s, min(C, total - s)))
        s += C
    return out


# -------------------------------------------------------------- device side

@with_exitstack
def _energy_body(ctx: ExitStack, tc, out_d, xh_d, mask_d, bands_d):
    nc = tc.nc
    Exp = mybir.ActivationFunctionType.Exp

    # ---- constants: ONE DMA for all band matrices, hi/lo split on device;
    # the folded set is materialized first (the first emitted unit needs it)
    consts = ctx.enter_context(tc.tile_pool(name="consts", bufs=1))
    bigb = consts.tile([128, 10 * 128], F32, name="bigb")
    nc.sync.dma_start(
        out=bigb.rearrange("p (i m) -> p i m", i=10),
        in_=bands_d.rearrange("i p m -> p i m"),
    )

    def load_bands(base, suffix):
        mhi, mlo = [], []
        for v in range(3):
            mf = bigb[:, (base + v) * 128 : (base + v + 1) * 128]
            hi = consts.tile([128, 128], F32R, name=f"mhi{suffix}{v}")
            nc.vector.tensor_copy(out=hi, in_=mf)
            mhi.append(hi)
            lo = consts.tile([128, 128], F32R, name=f"mlo{suffix}{v}")
            nc.vector.tensor_sub(out=lo, in0=mf, in1=hi)
            mlo.append(lo)
        bb = consts.tile([128, 128], F32R, name=f"bb{suffix}")
        nc.vector.tensor_copy(out=bb, in_=bigb[:, (base + 3) * 128 : (base + 4) * 128])
        bt = consts.tile([128, 128], F32R, name=f"bt{suffix}")
        nc.vector.tensor_copy(out=bt, in_=bigb[:, (base + 4) * 128 : (base + 5) * 128])
        return mhi, mlo, bb, bt

    MhiF, MloF, BBF, BTF = load_bands(5, "f")
    Mhi, Mlo, BB, BT = load_bands(0, "")
    SEGW = WH // 4

    xpool = ctx.enter_context(tc.tile_pool(name="xp", bufs=NBUFS))
    xhip = ctx.enter_context(tc.tile_pool(name="xhip", bufs=NBUFS))
    xlop = ctx.enter_context(tc.tile_pool(name="xlop", bufs=NBUFS))
    epool = ctx.enter_context(tc.tile_pool(name="ep", bufs=NBUFS))
    upool = ctx.enter_context(tc.tile_pool(name="up", bufs=NBUFS))
    rzpool = ctx.enter_context(tc.tile_pool(name="rzp", bufs=3))
    respool = ctx.enter_context(tc.tile_pool(name="resp", bufs=NBUFS))
    mpool = ctx.enter_context(tc.tile_pool(name="mp", bufs=2))
    ps_conv = ctx.enter_context(tc.tile_pool(name="psc", bufs=PS_BUFS, space="PSUM"))
    ps_z = ctx.enter_context(tc.tile_pool(name="psz", bufs=2, space="PSUM"))
    ps_s = ctx.enter_context(tc.tile_pool(name="pss", bufs=2, space="PSUM"))

    tiles = _make_tiles()

    def fold_unit(o, R, h):
        # Column-folded last row-tile: 4 width-segments of one half stacked
        # on 32-partition blocks, block-diagonal bands, ops span all 128
        # partitions (off-band lanes hold finite junk; masked exp gives
        # E=1 and the extended Z band keeps Z>0 there).
        mk = mpool.tile([128, 1], F32, tag="mk")
        nc.vector.memset(mk, 0.0)
        for b in range(4):
            nc.sync.dma_start(
                out=mk[32 * b : 32 * b + R + 4], in_=mask_d[o + 1 : o + R + 5, :]
            )
        if True:
            if True:
                g0 = h * WH
                X = xpool.tile([128, WH + 6], F32, tag="X")
                for b in range(4):
                    nc.sync.dma_start(
                        out=X[32 * b : 32 * b + 32, : SEGW + 6],
                        in_=xh_d[o : o + 32, g0 + b * SEGW : g0 + b * SEGW + SEGW + 6],
                    )
                Xhi = xhip.tile([128, WH + 6], F32R, tag="Xhi")
                nc.vector.tensor_copy(out=Xhi[:, : SEGW + 6], in_=X[:, : SEGW + 6])
                Xlo = xlop.tile([128, WH + 6], F32R, tag="Xlo")
                nc.vector.tensor_sub(
                    out=Xlo[:, : SEGW + 6],
                    in0=X[:, : SEGW + 6],
                    in1=Xhi[:, : SEGW + 6],
                )

                E = epool.tile([128, WH + 4], F32R, tag="E")
                for cs, cl in _chunks(SEGW + 4):
                    pc = ps_conv.tile([128, C], F32, tag="pc")
                    mms = []
                    for v in range(3):
                        mms.append((MhiF[v], Xhi, v))
                        mms.append((MloF[v], Xhi, v))
                    for v in range(3):
                        mms.append((MhiF[v], Xlo, v))
                    for i, (mband, xop, v) in enumerate(mms):
                        nc.tensor.matmul(
                            pc[:, :cl],
                            mband,
                            xop[:, cs + v : cs + v + cl],
                            start=(i == 0),
                            stop=(i == len(mms) - 1),
                        )
                    nc.scalar.activation(E[:, cs : cs + cl], pc[:, :cl], Exp, scale=mk)
                if h == 0:
                    nc.vector.memset(E[0:32, 0:2].bitcast(F32), 1.0)
                if h == WS - 1:
                    nc.vector.memset(E[96:128, SEGW + 2 : SEGW + 4].bitcast(F32), 1.0)

                U = upool.tile([128, WH + 2], F32R, tag="U")
                for cs, cl in _chunks(SEGW + 2):
                    pz = ps_z.tile([128, C], F32, tag="pz")
                    for v in range(3):
                        nc.tensor.matmul(
                            pz[:, :cl],
                            BTF,
                            E[:, cs + v : cs + v + cl],
                            start=(v == 0),
                            stop=(v == 2),
                        )
                    Rz = rzpool.tile([128, C], F32, tag="Rz")
                    nc.vector.reciprocal_approx_fast(out=Rz[:, :cl], in_=pz[:, :cl])
                    nc.vector.tensor_mul(
                        out=U[:, cs : cs + cl],
                        in0=X[:, cs + 2 : cs + 2 + cl],
                        in1=Rz[:, :cl],
                    )
                if h == 0:
                    nc.vector.memset(U[0:32, 0:1].bitcast(F32), 0.0)
                if h == WS - 1:
                    nc.vector.memset(U[96:128, SEGW + 1 : SEGW + 2].bitcast(F32), 0.0)

                res = respool.tile([128, WH], F32, tag="res")
                for cs, cl in _chunks(SEGW):
                    ps = ps_s.tile([128, C], F32, tag="ps")
                    for v in range(3):
                        nc.tensor.matmul(
                            ps[:, :cl],
                            BBF,
                            U[:, cs + v : cs + v + cl],
                            start=(v == 0),
                            stop=(v == 2),
                        )
                    nc.vector.tensor_mul(
                        out=res[:, cs : cs + cl],
                        in0=E[:, cs + 2 : cs + 2 + cl],
                        in1=ps[:, :cl],
                    )
                for b in range(4):
                    nc.sync.dma_start(
                        out=out_d[o : o + R, g0 + b * SEGW : g0 + (b + 1) * SEGW],
                        in_=res[32 * b + 2 : 32 * b + 2 + R, :SEGW],
                    )
            return

    def normal_tile(o, R):
        mk = mpool.tile([128, 1], F32, tag="mk")
        nc.sync.dma_start(out=mk[: R + 4], in_=mask_d[o + 1 : o + R + 5, :])
        for h in range(WS):
            g0 = h * WH
            # X[p, j] <-> (row r-3+p, global col g0-3+j)
            X = xpool.tile([128, WH + 6], F32, tag="X")
            nc.sync.dma_start(
                out=X[: R + 6, :], in_=xh_d[o : o + R + 6, g0 : g0 + WH + 6]
            )
            Xhi = xhip.tile([128, WH + 6], F32R, tag="Xhi")
            nc.vector.tensor_copy(out=Xhi[: R + 6, :], in_=X[: R + 6, :])
            Xlo = xlop.tile([128, WH + 6], F32R, tag="Xlo")
            nc.vector.tensor_sub(
                out=Xlo[: R + 6, :], in0=X[: R + 6, :], in1=Xhi[: R + 6, :]
            )

            # conv + exp -> E[m, e] <-> (row r-2+m, global col g0-2+e)
            E = epool.tile([128, WH + 4], F32R, tag="E")
            for cs, cl in _chunks(WH + 4):
                pc = ps_conv.tile([128, C], F32, tag="pc")
                mms = []
                for v in range(3):
                    mms.append((Mhi[v], Xhi, v))
                    mms.append((Mlo[v], Xhi, v))
                for v in range(3):
                    mms.append((Mhi[v], Xlo, v))
                for i, (mband, xop, v) in enumerate(mms):
                    nc.tensor.matmul(
                        pc[: R + 4, :cl],
                        mband[: R + 6, : R + 4],
                        xop[: R + 6, cs + v : cs + v + cl],
                        start=(i == 0),
                        stop=(i == len(mms) - 1),
                    )
                nc.scalar.activation(
                    E[: R + 4, cs : cs + cl],
                    pc[: R + 4, :cl],
                    Exp,
                    scale=mk[: R + 4],
                )
            # global-edge columns of E represent pad pixels: exp(0) = 1
            if h == 0:
                nc.vector.memset(E[: R + 4, 0:2].bitcast(F32), 1.0)
            if h == WS - 1:
                nc.vector.memset(E[: R + 4, WH + 2 : WH + 4].bitcast(F32), 1.0)

            # Z (vertical via BT, X frame) -> Rz -> U[m, z] (global col g0-1+z)
            U = upool.tile([128, WH + 2], F32R, tag="U")
            for cs, cl in _chunks(WH + 2):
                pz = ps_z.tile([128, C], F32, tag="pz")
                for v in range(3):
                    nc.tensor.matmul(
                        pz[: R + 4, :cl],
                        BT[: R + 4, : R + 4],
                        E[: R + 4, cs + v : cs + v + cl],
                        start=(v == 0),
                        stop=(v == 2),
                    )
                Rz = rzpool.tile([128, C], F32, tag="Rz")
                nc.vector.reciprocal_approx_fast(
                    out=Rz[: R + 4, :cl], in_=pz[: R + 4, :cl]
                )
                nc.vector.tensor_mul(
                    out=U[: R + 4, cs : cs + cl],
                    in0=X[: R + 4, cs + 2 : cs + 2 + cl],
                    in1=Rz[: R + 4, :cl],
                )
            # U at global-edge pad columns is 0 (fold drops OOB)
            if h == 0:
                nc.vector.memset(U[: R + 4, 0:1].bitcast(F32), 0.0)
            if h == WS - 1:
                nc.vector.memset(U[: R + 4, WH + 1 : WH + 2].bitcast(F32), 0.0)

            # S (vertical via BB, E frame) + res = E * S
            res = respool.tile([128, WH], F32, tag="res")
            for cs, cl in _chunks(WH):
                ps = ps_s.tile([128, C], F32, tag="ps")
                for v in range(3):
                    nc.tensor.matmul(
                        ps[: R + 2, :cl],
                        BB[: R + 4, : R + 2],
                        U[: R + 4, cs + v : cs + v + cl],
                        start=(v == 0),
                        stop=(v == 2),
                    )
                nc.vector.tensor_mul(
                    out=res[: R + 2, cs : cs + cl],
                    in0=E[: R + 2, cs + 2 : cs + 2 + cl],
                    in1=ps[: R + 2, :cl],
                )
            # valid output rows sit at partitions [2, R+2)
            nc.sync.dma_start(
                out=out_d[o : o + R, g0 : g0 + WH], in_=res[2 : R + 2, :WH]
            )

    of, Rf = tiles[-1]
    if len(tiles) > 1 and Rf <= 26:
        # cheap folded units at both pipeline edges: fast fill and drain
        fold_unit(of, Rf, 0)
        for o, R in tiles[:-1]:
            normal_tile(o, R)
        fold_unit(of, Rf, WS - 1)
    else:
        for o, R in tiles:
            normal_tile(o, R)


_CACHE: dict = {}


def _build():
    if "nc" in _CACHE:
        return _CACHE["nc"]
    nc = bacc.Bacc(
        "TRN2", target_bir_lowering=False, debug=False, num_devices=N_CORES
    )
    xh_d = nc.dram_tensor(
        "xh", (RC + 2 * HALO + 26, W + 2 * HALO), F32, kind="ExternalInput"
    ).ap()
    mask_d = nc.dram_tensor("mask", (RC + 2 * HALO, 1), F32, kind="ExternalInput").ap()
    bands_d = nc.dram_tensor("bands", (10, 128, 128), F32, kind="ExternalInput").ap()
    out_d = nc.dram_tensor("out", (RC, W), F32, kind="ExternalOutput").ap()
    with tile.TileContext(nc) as tc:
        _energy_body(tc, out_d, xh_d, mask_d, bands_d)
    nc.compile()
    _CACHE["nc"] = nc
    return nc


def kernel(shareable_energy: np.ndarray, kernel: np.ndarray, **_run_kw) -> np.ndarray:
    x = np.ascontiguousarray(np.asarray(shareable_energy, np.float32))
    k = np.asarray(kernel, np.float32)
    assert x.shape == (H, W), x.shape
    nc = _build()
    bands = _make_bands(k)
    in_maps = [_make_core_inputs(x, bands, core) for core in range(N_CORES)]
    r = run_bass_kernel_spmd(nc, in_maps, core_ids=list(range(N_CORES)), **_run_kw)
    out = np.concatenate([res["out"] for res in r.results], axis=0)
    if _run_kw:
        _CACHE["last_result"] = r
    return out



# revision 8
# speedup vs baseline: 1.0798x; 1.0798x over previous
"""Trainium2 Bass kernel for nn_EnergyDistributionCNN (3x3 conv -> unfold ->
softmax over patch -> weighted -> fold overlap-add), 8 NeuronCores.

Math (algebraically identical to the torch/jax reference):
    out = conv3x3(x, k)            cross-correlation, zero pad 1
    E   = exp(out)                 (pad pixels contribute exp(0)=1)
    Z   = boxsum3x3(E with ones at pad)
    U   = x / Z
    S   = boxsum3x3(U zero-padded)
    result = E * S

Sharding: row-block across 8 cores with a 3-row halo sliced on the host
(zero-filled at the global edges) -- no device-to-device communication.

Pipeline is fp16 end-to-end: the host casts x to fp16 (rel err 2.4e-4,
well inside the 2e-2 tolerance; measured end-to-end pipeline error is
~3e-3), which halves HBM traffic, runs the PE at full rate and unlocks
the DVE 2x packed-16-bit mode.  Work is spread over all five engines:

  PE   : conv (3 band-matmul passes), Z box (3 shifted ones-band passes,
         or 1 pass on 'mix' tiles), S vertical (1 pass)
  Act  : exp from conv's PSUM (with the row-validity mask folded into the
         scale operand: exp(0*junk)=1), and the S PSUM->fp16 copy
  DVE  : U = x/Z as a single tensor_tensor divide straight from PSUM,
         second horizontal add of the U box, final res = E*S fp16 mul,
         (on 'mix' tiles also the horizontal E adds)
  Pool : first horizontal add of the U box (GpSimd is otherwise idle)
  DMA  : fp16 loads/stores

Row mapping keeps every compute op at partition base 0 (hardware only
allows compute APs to start at partitions 0/32/64/96): the banded
matrices alternate lower/upper diagonals so each stage's output lands
re-centred, and frame-edge partitions hold junk that is either masked
(exp scale), harmless (finite, unused), or skipped by the output DMA
(which may start at any partition).
"""

from contextlib import ExitStack

import numpy as np

import concourse.bacc as bacc
import concourse.mybir as mybir
import concourse.tile as tile
from concourse._compat import with_exitstack
from concourse.bass_utils import run_bass_kernel_spmd
from concourse.dve_ops import RECIP_APPROX_FAST_CONSTS, RECIPROCAL_APPROX_FAST

F16 = mybir.dt.float16
F32 = mybir.dt.float32

H = 4096
W = 4096
N_CORES = 8
RC = H // N_CORES  # rows per core
HALO = 3
RT = 122           # output rows per normal row-tile (RT + 6 <= 128)
C = 512            # matmul column chunk = one fp32 PSUM bank
CG = 1024          # conv PSUM group (2 banks) -> fewer Act instructions
SEGW = 1024        # folded-tile width segment (4 segs on 32-row blocks)

# Which normal tiles compute the Z box via Eh-on-DVE + 1 matmul instead of
# 3 shifted matmuls on the PE (engine balancing knob).
ZMIX = (False, True, False, False)

XW = W + 2 * HALO        # X cols:  j   <-> global col j-3   (4102)
EW = W + 4               # E cols:  e   <-> global col e-2   (4100)
ZW = W + 2               # Z/U cols: c  <-> global col c-1   (4098)


# ---------------------------------------------------------------- host side

def _band(vals, lo):
    """128x128 band matrix: b[p, m] = vals[p-m-lo] for p-m-lo in 0..2."""
    b = np.zeros((128, 128), np.float32)
    idx = np.arange(128)
    for d in range(3):
        off = lo + d
        p = idx[off:] if off >= 0 else idx[: 128 + off]
        m = p - off
        b[p, m] = vals[d]
    return b


def _make_bands(k: np.ndarray) -> np.ndarray:
    """bands[0..2]: conv lhsT per column-shift v (b[p,m]=k[p-m, v]);
    bands[3]: BT ones, lhsT[m,p]=1 for p-m in 0..2 (Z: E-frame -> X-frame);
    bands[4]: BS ones, lhsT[p,m]=1 for p-m in 0..2 (S: X-frame -> E-frame);
    bands[5..9]: the same five as 4x 32x32 block-diagonals (folded tile)."""
    bands = np.zeros((10, 128, 128), np.float32)
    for v in range(3):
        bands[v] = _band(k[:, v], 0)
    bands[3] = _band([1.0, 1.0, 1.0], 0).T
    bands[4] = _band([1.0, 1.0, 1.0], 0)
    for i in range(5):
        for b in range(4):
            s = slice(32 * b, 32 * b + 32)
            bands[5 + i][s, s] = bands[i][:32, :32]
    return bands.astype(np.float16)


def _make_core_inputs(x16: np.ndarray, bands: np.ndarray, core: int):
    r0 = core * RC
    lo, hi = r0 - HALO, r0 + RC + HALO
    xh = np.zeros((RC + 2 * HALO, XW), np.float16)
    s_lo, s_hi = max(lo, 0), min(hi, H)
    xh[s_lo - lo : s_hi - lo, HALO : HALO + W] = x16[s_lo:s_hi]
    gl = np.arange(lo, hi)
    mask = ((gl >= 0) & (gl < H)).astype(np.float32)[:, None]
    return {"xh": xh, "mask": mask, "bands": bands}


def _chunks(total: int, step: int):
    out = []
    s = 0
    while s < total:
        out.append((s, min(step, total - s)))
        s += step
    return out


# -------------------------------------------------------------- device side

@with_exitstack
def _energy_body(ctx: ExitStack, tc, out_d, xh_d, mask_d, bands_d):
    nc = tc.nc
    Exp = mybir.ActivationFunctionType.Exp
    Copy = mybir.ActivationFunctionType.Copy
    RC_ = RECIP_APPROX_FAST_CONSTS

    def recip16(out_ap, in_ap):
        # reciprocal_approx_fast with an fp16 output (the DVE output stage
        # downconverts; the fp32 bit-trick only concerns the input)
        nc.vector._custom_dve(
            RECIPROCAL_APPROX_FAST, out=out_ap, in0=in_ap,
            s0=RC_["s0"], s1=RC_["s1"], imm2=RC_["imm2"],
        )

    consts = ctx.enter_context(tc.tile_pool(name="consts", bufs=1))
    bigb = consts.tile([128, 10 * 128], F16, name="bigb")
    nc.sync.dma_start(
        out=bigb.rearrange("p (i m) -> p i m", i=10),
        in_=bands_d.rearrange("i p m -> p i m"),
    )
    Mv = [bigb[:, i * 128 : (i + 1) * 128] for i in range(3)]
    BT = bigb[:, 3 * 128 : 4 * 128]
    BS = bigb[:, 4 * 128 : 5 * 128]
    MvF = [bigb[:, (5 + i) * 128 : (6 + i) * 128] for i in range(3)]
    BTF = bigb[:, 8 * 128 : 9 * 128]
    BSF = bigb[:, 9 * 128 : 10 * 128]

    xpool = ctx.enter_context(tc.tile_pool(name="xp", bufs=2))
    epool = ctx.enter_context(tc.tile_pool(name="ep", bufs=2))
    ehpool = ctx.enter_context(tc.tile_pool(name="ehp", bufs=2))
    upool = ctx.enter_context(tc.tile_pool(name="up", bufs=2))
    uhpool = ctx.enter_context(tc.tile_pool(name="uhp", bufs=2))
    spool = ctx.enter_context(tc.tile_pool(name="sp", bufs=2))
    respool = ctx.enter_context(tc.tile_pool(name="resp", bufs=2))
    mpool = ctx.enter_context(tc.tile_pool(name="mp", bufs=2))
    ps_c = ctx.enter_context(tc.tile_pool(name="psc", bufs=2, space="PSUM"))
    ps_z = ctx.enter_context(tc.tile_pool(name="psz", bufs=2, space="PSUM"))
    ps_s = ctx.enter_context(tc.tile_pool(name="pss", bufs=2, space="PSUM"))

    def normal_tile(o, R, zmix):
        P = R + 4  # working partitions (E frame); X uses R+6
        mk = mpool.tile([128, 1], F32, tag="mk")
        nc.sync.dma_start(out=mk[:P], in_=mask_d[o + 1 : o + 1 + P, :])

        X = xpool.tile([128, XW], F16, tag="X")
        nc.sync.dma_start(out=X[: R + 6, :], in_=xh_d[o : o + R + 6, :])

        # conv + exp -> E[m, e] <-> (row o-2+m, col e-2)
        E = epool.tile([128, EW], F16, tag="E")
        for g0, gl in _chunks(EW, CG):
            pc = ps_c.tile([128, CG], F32, tag="pc")
            for cs, cl in _chunks(gl, C):
                for v in range(3):
                    nc.tensor.matmul(
                        pc[:P, cs : cs + cl],
                        Mv[v][: R + 6, :P],
                        X[: R + 6, g0 + cs + v : g0 + cs + v + cl],
                        start=(v == 0),
                        stop=(v == 2),
                    )
            nc.scalar.activation(E[:P, g0 : g0 + gl], pc[:P, :gl], Exp, scale=mk[:P])
        # pad columns of E represent out-of-grid pixels: exp(0) = 1
        nc.vector.memset(E[:P, 1:2], 1.0)
        nc.vector.memset(E[:P, EW - 2 : EW - 1], 1.0)

        if zmix:
            # horizontal E box on DVE, vertical on PE (1 pass)
            eh1 = ehpool.tile([128, ZW], F16, tag="eh1")
            nc.vector.tensor_add(out=eh1[:P, :], in0=E[:P, 0:ZW], in1=E[:P, 1 : ZW + 1])
            eh = ehpool.tile([128, ZW], F16, tag="eh")
            nc.vector.tensor_add(out=eh[:P, :], in0=eh1[:P, :], in1=E[:P, 2 : ZW + 2])

        # Z (X frame via BT), Rz = 1/Z from PSUM, then U = x * Rz in fp16
        Rz = ehpool.tile([128, ZW], F16, tag="Rz")
        for cs, cl in _chunks(ZW, C):
            pz = ps_z.tile([128, C], F32, tag="pz")
            if zmix:
                nc.tensor.matmul(
                    pz[:P, :cl], BT[:P, :P], eh[:P, cs : cs + cl],
                    start=True, stop=True,
                )
            else:
                for v in range(3):
                    nc.tensor.matmul(
                        pz[:P, :cl], BT[:P, :P], E[:P, cs + v : cs + v + cl],
                        start=(v == 0), stop=(v == 2),
                    )
            recip16(Rz[:P, cs : cs + cl], pz[:P, :cl])
        U = upool.tile([128, ZW], F16, tag="U")
        nc.vector.tensor_mul(out=U[:P, :], in0=X[:P, 2 : ZW + 2], in1=Rz[:P, :])

        # horizontal U box: first add on GpSimd (otherwise idle), second DVE
        uh1 = uhpool.tile([128, W], F16, tag="uh1")
        nc.gpsimd.tensor_add(out=uh1[:P, :], in0=U[:P, 0:W], in1=U[:P, 1 : W + 1])
        uh = uhpool.tile([128, W], F16, tag="uh")
        nc.vector.tensor_add(out=uh[:P, :], in0=uh1[:P, :], in1=U[:P, 2 : W + 2])

        # S vertical (E frame via BS) -> fp16 via Act copy -> res = E*S
        S16 = spool.tile([128, W], F16, tag="S16")
        for cs, cl in _chunks(W, C):
            ps = ps_s.tile([128, C], F32, tag="ps")
            nc.tensor.matmul(
                ps[: R + 2, :cl], BS[:P, : R + 2], uh[:P, cs : cs + cl],
                start=True, stop=True,
            )
            nc.scalar.activation(S16[: R + 2, cs : cs + cl], ps[: R + 2, :cl], Copy)
        res = respool.tile([128, W], F16, tag="res")
        nc.vector.tensor_mul(
            out=res[: R + 2, :], in0=E[: R + 2, 2 : W + 2], in1=S16[: R + 2, :]
        )
        # valid output rows sit at partitions [2, R+2)
        nc.sync.dma_start(out=out_d[o : o + R, :], in_=res[2 : R + 2, :W])

    def fold_tile(o, R):
        # Last 24 rows: 4 width segments of 1024 stacked on 32-partition
        # blocks, block-diagonal bands.  Off-band lanes hold finite junk
        # (masked exp gives E=1 there, X junk rows divide to finite U).
        mk = mpool.tile([128, 1], F32, tag="mk")
        nc.vector.memset(mk, 0.0)
        for b in range(4):
            nc.sync.dma_start(
                out=mk[32 * b : 32 * b + R + 4], in_=mask_d[o + 1 : o + R + 5, :]
            )
        X = xpool.tile([128, SEGW + 6], F16, tag="X")
        for b in range(4):
            nc.sync.dma_start(
                out=X[32 * b : 32 * b + R + 6, :],
                in_=xh_d[o : o + R + 6, b * SEGW : b * SEGW + SEGW + 6],
            )

        ew, zw = SEGW + 4, SEGW + 2
        E = epool.tile([128, ew], F16, tag="E")
        for g0, gl in _chunks(ew, CG):
            pc = ps_c.tile([128, CG], F32, tag="pc")
            for cs, cl in _chunks(gl, C):
                for v in range(3):
                    nc.tensor.matmul(
                        pc[:, cs : cs + cl],
                        MvF[v],
                        X[:, g0 + cs + v : g0 + cs + v + cl],
                        start=(v == 0),
                        stop=(v == 2),
                    )
            nc.scalar.activation(E[:, g0 : g0 + gl], pc[:, :gl], Exp, scale=mk)
        nc.vector.memset(E[0:32, 1:2], 1.0)
        nc.vector.memset(E[96:128, ew - 2 : ew - 1], 1.0)

        Rz = ehpool.tile([128, zw], F16, tag="Rz")
        for cs, cl in _chunks(zw, C):
            pz = ps_z.tile([128, C], F32, tag="pz")
            for v in range(3):
                nc.tensor.matmul(
                    pz[:, :cl], BTF, E[:, cs + v : cs + v + cl],
                    start=(v == 0), stop=(v == 2),
                )
            recip16(Rz[:, cs : cs + cl], pz[:, :cl])
        U = upool.tile([128, zw], F16, tag="U")
        nc.vector.tensor_mul(out=U, in0=X[:, 2 : zw + 2], in1=Rz)

        uh1 = uhpool.tile([128, SEGW], F16, tag="uh1")
        nc.gpsimd.tensor_add(out=uh1, in0=U[:, 0:SEGW], in1=U[:, 1 : SEGW + 1])
        uh = uhpool.tile([128, SEGW], F16, tag="uh")
        nc.vector.tensor_add(out=uh, in0=uh1, in1=U[:, 2 : SEGW + 2])

        S16 = spool.tile([128, SEGW], F16, tag="S16")
        for cs, cl in _chunks(SEGW, C):
            ps = ps_s.tile([128, C], F32, tag="ps")
            nc.tensor.matmul(ps[:, :cl], BSF, uh[:, cs : cs + cl], start=True, stop=True)
            nc.scalar.activation(S16[:, cs : cs + cl], ps[:, :cl], Copy)
        res = respool.tile([128, SEGW], F16, tag="res")
        nc.vector.tensor_mul(out=res, in0=E[:, 2 : SEGW + 2], in1=S16)
        for b in range(4):
            nc.sync.dma_start(
                out=out_d[o : o + R, b * SEGW : (b + 1) * SEGW],
                in_=res[32 * b + 2 : 32 * b + 2 + R, :],
            )

    with nc.allow_low_precision("fp16 pipeline; verified within tolerance"):
        tiles = _chunks(RC, RT)
        fo, fr = tiles[-1]
        normal = tiles[:-1] if fr <= 26 else tiles
        for i, (o, R) in enumerate(normal):
            normal_tile(o, R, ZMIX[i % len(ZMIX)])
        if fr <= 26:
            fold_tile(fo, fr)


_CACHE: dict = {}


def _build():
    if "nc" in _CACHE:
        return _CACHE["nc"]
    nc = bacc.Bacc(
        "TRN2", target_bir_lowering=False, debug=False, num_devices=N_CORES
    )
    xh_d = nc.dram_tensor("xh", (RC + 2 * HALO, XW), F16, kind="ExternalInput").ap()
    mask_d = nc.dram_tensor("mask", (RC + 2 * HALO, 1), F32, kind="ExternalInput").ap()
    bands_d = nc.dram_tensor("bands", (10, 128, 128), F16, kind="ExternalInput").ap()
    out_d = nc.dram_tensor("out", (RC, W), F16, kind="ExternalOutput").ap()
    with tile.TileContext(nc) as tc:
        _energy_body(tc, out_d, xh_d, mask_d, bands_d)
    nc.compile()
    _CACHE["nc"] = nc
    return nc


def kernel(shareable_energy: np.ndarray, kernel: np.ndarray, **_run_kw) -> np.ndarray:
    x = np.asarray(shareable_energy, np.float32)
    k = np.asarray(kernel, np.float32)
    assert x.shape == (H, W), x.shape
    nc = _build()
    x16 = x.astype(np.float16)
    bands = _make_bands(k)
    in_maps = [_make_core_inputs(x16, bands, core) for core in range(N_CORES)]
    r = run_bass_kernel_spmd(nc, in_maps, core_ids=list(range(N_CORES)), **_run_kw)
    out = np.concatenate(
        [res["out"].astype(np.float32) for res in r.results], axis=0
    )
    if _run_kw:
        _CACHE["last_result"] = r
    return out


# revision 12
# speedup vs baseline: 1.4010x; 1.2975x over previous
"""Trainium2 Bass kernel for nn_EnergyDistributionCNN (3x3 conv -> unfold ->
softmax over patch -> weighted -> fold overlap-add), 8 NeuronCores.

Math (algebraically identical to the torch/jax reference):
    out = conv3x3(x, k)            cross-correlation, zero pad 1
    E   = exp(out)                 (pad pixels contribute exp(0)=1)
    Z   = boxsum3x3(E with ones at pad)
    U   = x / Z
    S   = boxsum3x3(U zero-padded)
    result = E * S

Sharding: row-block across 8 cores with a 3-row halo sliced on the host
(zero-filled at the global edges) -- no device-to-device communication.

Pipeline is fp16 end-to-end: the host casts x to fp16 (rel err 2.4e-4,
well inside the 2e-2 tolerance; measured end-to-end pipeline error is
~3e-3), which halves HBM traffic, runs the PE at full rate and unlocks
the DVE 2x packed-16-bit mode.  Work is spread over all five engines:

  PE   : conv (3 band-matmul passes), Z box (3 shifted ones-band passes,
         or 1 pass on 'mix' tiles), S vertical (1 pass)
  Act  : exp from conv's PSUM (with the row-validity mask folded into the
         scale operand: exp(0*junk)=1), and the S PSUM->fp16 copy
  DVE  : U = x/Z as a single tensor_tensor divide straight from PSUM,
         second horizontal add of the U box, final res = E*S fp16 mul,
         (on 'mix' tiles also the horizontal E adds)
  Pool : first horizontal add of the U box (GpSimd is otherwise idle)
  DMA  : fp16 loads/stores

Row mapping keeps every compute op at partition base 0 (hardware only
allows compute APs to start at partitions 0/32/64/96): the banded
matrices alternate lower/upper diagonals so each stage's output lands
re-centred, and frame-edge partitions hold junk that is either masked
(exp scale), harmless (finite, unused), or skipped by the output DMA
(which may start at any partition).
"""

from contextlib import ExitStack

import numpy as np

import concourse.bacc as bacc
import concourse.mybir as mybir
import concourse.tile as tile
from concourse._compat import with_exitstack
from concourse.bass_utils import run_bass_kernel_spmd
from concourse.dve_ops import RECIP_APPROX_FAST_CONSTS, RECIPROCAL_APPROX_FAST

F16 = mybir.dt.float16
F32 = mybir.dt.float32

H = 4096
W = 4096
N_CORES = 8
RC = H // N_CORES  # rows per core
HALO = 3
RT = 122           # output rows per normal row-tile (RT + 6 <= 128)
C = 512            # matmul column chunk = one fp32 PSUM bank
CG = 1024          # conv PSUM group (2 banks) -> fewer Act instructions
SEGW = 1024        # folded-tile width segment (4 segs on 32-row blocks)

# Which normal tiles compute the Z box via Eh-on-DVE + 1 matmul instead of
# 3 shifted matmuls on the PE (engine balancing knob).
ZMIX = (False, False, False, False)

XW = W + 2 * HALO        # X cols:  j   <-> global col j-3   (4102)
EW = W + 4               # E cols:  e   <-> global col e-2   (4100)
ZW = W + 2               # Z/U cols: c  <-> global col c-1   (4098)


# ---------------------------------------------------------------- host side

def _band(vals, lo):
    """128x128 band matrix: b[p, m] = vals[p-m-lo] for p-m-lo in 0..2."""
    b = np.zeros((128, 128), np.float32)
    idx = np.arange(128)
    for d in range(3):
        off = lo + d
        p = idx[off:] if off >= 0 else idx[: 128 + off]
        m = p - off
        b[p, m] = vals[d]
    return b


def _make_bands(k: np.ndarray) -> np.ndarray:
    """bands[0..2]: conv lhsT per column-shift v (b[p,m]=k[p-m, v]);
    bands[3]: BT ones, lhsT[m,p]=1 for p-m in 0..2 (Z: E-frame -> X-frame);
    bands[4]: BS ones, lhsT[p,m]=1 for p-m in 0..2 (S: X-frame -> E-frame);
    bands[5..9]: the same five as 4x 32x32 block-diagonals (folded tile)."""
    bands = np.zeros((10, 128, 128), np.float32)
    for v in range(3):
        bands[v] = _band(k[:, v], 0)
    bands[3] = _band([1.0, 1.0, 1.0], 0).T
    bands[4] = _band([1.0, 1.0, 1.0], 0)
    for i in range(5):
        for b in range(4):
            s = slice(32 * b, 32 * b + 32)
            bands[5 + i][s, s] = bands[i][:32, :32]
    return bands.astype(np.float16)


def _make_core_inputs(x16: np.ndarray, bands: np.ndarray, core: int):
    r0 = core * RC
    lo, hi = r0 - HALO, r0 + RC + HALO
    xh = np.zeros((RC + 2 * HALO, XW), np.float16)
    s_lo, s_hi = max(lo, 0), min(hi, H)
    xh[s_lo - lo : s_hi - lo, HALO : HALO + W] = x16[s_lo:s_hi]
    gl = np.arange(lo, hi)
    mask = ((gl >= 0) & (gl < H)).astype(np.float32)[:, None]
    return {"xh": xh, "mask": mask, "bands": bands}


def _chunks(total: int, step: int):
    out = []
    s = 0
    while s < total:
        out.append((s, min(step, total - s)))
        s += step
    return out


# -------------------------------------------------------------- device side

@with_exitstack
def _energy_body(ctx: ExitStack, tc, out_d, xh_d, mask_d, bands_d):
    nc = tc.nc
    Exp = mybir.ActivationFunctionType.Exp
    Copy = mybir.ActivationFunctionType.Copy
    RC_ = RECIP_APPROX_FAST_CONSTS

    def recip16(out_ap, in_ap):
        # reciprocal_approx_fast with an fp16 output (the DVE output stage
        # downconverts; the fp32 bit-trick only concerns the input)
        nc.vector._custom_dve(
            RECIPROCAL_APPROX_FAST, out=out_ap, in0=in_ap,
            s0=RC_["s0"], s1=RC_["s1"], imm2=RC_["imm2"],
        )

    consts = ctx.enter_context(tc.tile_pool(name="consts", bufs=1))
    bigb = consts.tile([128, 10 * 128], F16, name="bigb")
    nc.sync.dma_start(
        out=bigb.rearrange("p (i m) -> p i m", i=10),
        in_=bands_d.rearrange("i p m -> p i m"),
    )
    Mv = [bigb[:, i * 128 : (i + 1) * 128] for i in range(3)]
    BT = bigb[:, 3 * 128 : 4 * 128]
    BS = bigb[:, 4 * 128 : 5 * 128]
    MvF = [bigb[:, (5 + i) * 128 : (6 + i) * 128] for i in range(3)]
    BTF = bigb[:, 8 * 128 : 9 * 128]
    BSF = bigb[:, 9 * 128 : 10 * 128]

    xpool = ctx.enter_context(tc.tile_pool(name="xp", bufs=2))
    epool = ctx.enter_context(tc.tile_pool(name="ep", bufs=2))
    ehpool = ctx.enter_context(tc.tile_pool(name="ehp", bufs=3))
    upool = ctx.enter_context(tc.tile_pool(name="up", bufs=3))
    uhpool = ctx.enter_context(tc.tile_pool(name="uhp", bufs=3))
    spool = ctx.enter_context(tc.tile_pool(name="sp", bufs=3))
    respool = ctx.enter_context(tc.tile_pool(name="resp", bufs=3))
    mpool = ctx.enter_context(tc.tile_pool(name="mp", bufs=2))
    ps_c = ctx.enter_context(tc.tile_pool(name="psc", bufs=2, space="PSUM"))
    ps_z = ctx.enter_context(tc.tile_pool(name="psz", bufs=2, space="PSUM"))
    ps_s = ctx.enter_context(tc.tile_pool(name="pss", bufs=2, space="PSUM"))

    def normal_tile(o, R, zmix):
        P = R + 4  # working partitions (E frame); X uses R+6
        mk = mpool.tile([128, 1], F32, tag="mk")
        nc.sync.dma_start(out=mk[:P], in_=mask_d[o + 1 : o + 1 + P, :])

        X = xpool.tile([128, XW], F16, tag="X")
        nc.sync.dma_start(out=X[: R + 6, :], in_=xh_d[o : o + R + 6, :])

        # conv + exp -> E[m, e] <-> (row o-2+m, col e-2)
        E = epool.tile([128, EW], F16, tag="E")
        for g0, gl in _chunks(EW, CG):
            pc = ps_c.tile([128, CG], F32, tag="pc")
            for cs, cl in _chunks(gl, C):
                for v in range(3):
                    nc.tensor.matmul(
                        pc[:P, cs : cs + cl],
                        Mv[v][: R + 6, :P],
                        X[: R + 6, g0 + cs + v : g0 + cs + v + cl],
                        start=(v == 0),
                        stop=(v == 2),
                    )
            nc.scalar.activation(E[:P, g0 : g0 + gl], pc[:P, :gl], Exp, scale=mk[:P])
        # pad columns of E represent out-of-grid pixels: exp(0) = 1
        nc.vector.memset(E[:P, 1:2], 1.0)
        nc.vector.memset(E[:P, EW - 2 : EW - 1], 1.0)

        if zmix:
            # horizontal E box on DVE, vertical on PE (1 pass)
            eh1 = ehpool.tile([128, ZW], F16, tag="eh1")
            nc.vector.tensor_add(out=eh1[:P, :], in0=E[:P, 0:ZW], in1=E[:P, 1 : ZW + 1])
            eh = ehpool.tile([128, ZW], F16, tag="eh")
            nc.vector.tensor_add(out=eh[:P, :], in0=eh1[:P, :], in1=E[:P, 2 : ZW + 2])

        # Z (X frame via BT), Rz = 1/Z from PSUM, then U = x * Rz in fp16
        Rz = ehpool.tile([128, ZW], F16, tag="Rz")
        for cs, cl in _chunks(ZW, C):
            pz = ps_z.tile([128, C], F32, tag="pz")
            if zmix:
                nc.tensor.matmul(
                    pz[:P, :cl], BT[:P, :P], eh[:P, cs : cs + cl],
                    start=True, stop=True,
                )
            else:
                for v in range(3):
                    nc.tensor.matmul(
                        pz[:P, :cl], BT[:P, :P], E[:P, cs + v : cs + v + cl],
                        start=(v == 0), stop=(v == 2),
                    )
            recip16(Rz[:P, cs : cs + cl], pz[:P, :cl])
        # U = x * Rz, horizontal U box (first add on the otherwise-idle
        # GpSimd, second on DVE), S vertical (E frame via BS), PSUM -> fp16
        # via Act copy, res = E*S.  All split into column halves so the
        # engines pipeline within a tile instead of serializing on
        # full-width tensors.
        U = upool.tile([128, ZW], F16, tag="U")
        uh1 = uhpool.tile([128, W], F16, tag="uh1")
        uh = uhpool.tile([128, W], F16, tag="uh")
        S16 = spool.tile([128, W], F16, tag="S16")
        res = respool.tile([128, W], F16, tag="res")
        HW_ = W // 2
        for h0 in (0, HW_):
            # U half 0 covers cols [0, HW_+2), half 1 the remaining [HW_+2, ZW)
            u0, ul = (0, HW_ + 2) if h0 == 0 else (HW_ + 2, ZW - HW_ - 2)
            nc.vector.tensor_mul(
                out=U[:P, u0 : u0 + ul],
                in0=X[:P, u0 + 2 : u0 + 2 + ul],
                in1=Rz[:P, u0 : u0 + ul],
            )
            nc.gpsimd.tensor_add(
                out=uh1[:P, h0 : h0 + HW_],
                in0=U[:P, h0 : h0 + HW_],
                in1=U[:P, h0 + 1 : h0 + 1 + HW_],
            )
            nc.vector.tensor_add(
                out=uh[:P, h0 : h0 + HW_],
                in0=uh1[:P, h0 : h0 + HW_],
                in1=U[:P, h0 + 2 : h0 + 2 + HW_],
            )
            for cs, cl in _chunks(HW_, C):
                ps = ps_s.tile([128, C], F32, tag="ps")
                nc.tensor.matmul(
                    ps[: R + 2, :cl], BS[:P, : R + 2],
                    uh[:P, h0 + cs : h0 + cs + cl],
                    start=True, stop=True,
                )
                nc.scalar.activation(
                    S16[: R + 2, h0 + cs : h0 + cs + cl], ps[: R + 2, :cl], Copy
                )
            nc.vector.tensor_mul(
                out=res[: R + 2, h0 : h0 + HW_],
                in0=E[: R + 2, h0 + 2 : h0 + 2 + HW_],
                in1=S16[: R + 2, h0 : h0 + HW_],
            )
            # valid output rows sit at partitions [2, R+2)
            nc.sync.dma_start(
                out=out_d[o : o + R, h0 : h0 + HW_],
                in_=res[2 : R + 2, h0 : h0 + HW_],
            )

    def fold_tile(o, R):
        # Last 24 rows: 4 width segments of 1024 stacked on 32-partition
        # blocks, block-diagonal bands.  Off-band lanes hold finite junk
        # (masked exp gives E=1 there, X junk rows divide to finite U).
        mk = mpool.tile([128, 1], F32, tag="mk")
        nc.vector.memset(mk, 0.0)
        for b in range(4):
            nc.sync.dma_start(
                out=mk[32 * b : 32 * b + R + 4], in_=mask_d[o + 1 : o + R + 5, :]
            )
        X = xpool.tile([128, SEGW + 6], F16, tag="X")
        for b in range(4):
            nc.sync.dma_start(
                out=X[32 * b : 32 * b + R + 6, :],
                in_=xh_d[o : o + R + 6, b * SEGW : b * SEGW + SEGW + 6],
            )

        ew, zw = SEGW + 4, SEGW + 2
        E = epool.tile([128, ew], F16, tag="E")
        for g0, gl in _chunks(ew, CG):
            pc = ps_c.tile([128, CG], F32, tag="pc")
            for cs, cl in _chunks(gl, C):
                for v in range(3):
                    nc.tensor.matmul(
                        pc[:, cs : cs + cl],
                        MvF[v],
                        X[:, g0 + cs + v : g0 + cs + v + cl],
                        start=(v == 0),
                        stop=(v == 2),
                    )
            nc.scalar.activation(E[:, g0 : g0 + gl], pc[:, :gl], Exp, scale=mk)
        nc.vector.memset(E[0:32, 1:2], 1.0)
        nc.vector.memset(E[96:128, ew - 2 : ew - 1], 1.0)

        Rz = ehpool.tile([128, zw], F16, tag="Rz")
        for cs, cl in _chunks(zw, C):
            pz = ps_z.tile([128, C], F32, tag="pz")
            for v in range(3):
                nc.tensor.matmul(
                    pz[:, :cl], BTF, E[:, cs + v : cs + v + cl],
                    start=(v == 0), stop=(v == 2),
                )
            recip16(Rz[:, cs : cs + cl], pz[:, :cl])
        U = upool.tile([128, zw], F16, tag="U")
        nc.vector.tensor_mul(out=U, in0=X[:, 2 : zw + 2], in1=Rz)

        uh1 = uhpool.tile([128, SEGW], F16, tag="uh1")
        nc.gpsimd.tensor_add(out=uh1, in0=U[:, 0:SEGW], in1=U[:, 1 : SEGW + 1])
        uh = uhpool.tile([128, SEGW], F16, tag="uh")
        nc.vector.tensor_add(out=uh, in0=uh1, in1=U[:, 2 : SEGW + 2])

        S16 = spool.tile([128, SEGW], F16, tag="S16")
        for cs, cl in _chunks(SEGW, C):
            ps = ps_s.tile([128, C], F32, tag="ps")
            nc.tensor.matmul(ps[:, :cl], BSF, uh[:, cs : cs + cl], start=True, stop=True)
            nc.scalar.activation(S16[:, cs : cs + cl], ps[:, :cl], Copy)
        res = respool.tile([128, SEGW], F16, tag="res")
        nc.vector.tensor_mul(out=res, in0=E[:, 2 : SEGW + 2], in1=S16)
        for b in range(4):
            nc.sync.dma_start(
                out=out_d[o : o + R, b * SEGW : (b + 1) * SEGW],
                in_=res[32 * b + 2 : 32 * b + 2 + R, :],
            )

    with nc.allow_low_precision("fp16 pipeline; verified within tolerance"):
        tiles = _chunks(RC, RT)
        fo, fr = tiles[-1]
        normal = tiles[:-1] if fr <= 26 else tiles
        for i, (o, R) in enumerate(normal):
            normal_tile(o, R, ZMIX[i % len(ZMIX)])
        if fr <= 26:
            fold_tile(fo, fr)


_CACHE: dict = {}


def _build():
    if "nc" in _CACHE:
        return _CACHE["nc"]
    nc = bacc.Bacc(
        "TRN2", target_bir_lowering=False, debug=False, num_devices=N_CORES
    )
    xh_d = nc.dram_tensor("xh", (RC + 2 * HALO, XW), F16, kind="ExternalInput").ap()
    mask_d = nc.dram_tensor("mask", (RC + 2 * HALO, 1), F32, kind="ExternalInput").ap()
    bands_d = nc.dram_tensor("bands", (10, 128, 128), F16, kind="ExternalInput").ap()
    out_d = nc.dram_tensor("out", (RC, W), F16, kind="ExternalOutput").ap()
    with tile.TileContext(nc) as tc:
        _energy_body(tc, out_d, xh_d, mask_d, bands_d)
    nc.compile()
    _CACHE["nc"] = nc
    return nc


def kernel(shareable_energy: np.ndarray, kernel: np.ndarray, **_run_kw) -> np.ndarray:
    x = np.asarray(shareable_energy, np.float32)
    k = np.asarray(kernel, np.float32)
    assert x.shape == (H, W), x.shape
    nc = _build()
    x16 = x.astype(np.float16)
    bands = _make_bands(k)
    in_maps = [_make_core_inputs(x16, bands, core) for core in range(N_CORES)]
    r = run_bass_kernel_spmd(nc, in_maps, core_ids=list(range(N_CORES)), **_run_kw)
    out = np.concatenate(
        [res["out"].astype(np.float32) for res in r.results], axis=0
    )
    if _run_kw:
        _CACHE["last_result"] = r
    return out


# revision 24
# speedup vs baseline: 1.4789x; 1.0556x over previous
"""Trainium2 Bass kernel for nn_EnergyDistributionCNN (3x3 conv -> unfold ->
softmax over patch -> weighted -> fold overlap-add), 8 NeuronCores.

Math (algebraically identical to the torch/jax reference):
    out = conv3x3(x, k)            cross-correlation, zero pad 1
    E   = exp(out)                 (pad pixels contribute exp(0)=1)
    Z   = boxsum3x3(E with ones at pad)
    U   = x / Z
    S   = boxsum3x3(U zero-padded)
    result = E * S

Sharding: row-block across 8 cores with a 3-row halo sliced on the host
(zero-filled at the global edges) -- no device-to-device communication.

Pipeline is fp16 end-to-end: the host casts x to fp16 (rel err 2.4e-4,
well inside the 2e-2 tolerance; measured end-to-end pipeline error is
~3e-3), which halves HBM traffic, runs the PE at full rate and unlocks
the DVE 2x packed-16-bit mode.  Work is spread over all five engines:

  PE   : conv (3 band-matmul passes), Z box (3 shifted ones-band passes,
         or 1 pass on 'mix' tiles), S vertical (1 pass)
  Act  : exp from conv's PSUM (with the row-validity mask folded into the
         scale operand: exp(0*junk)=1), and the S PSUM->fp16 copy
  DVE  : U = x/Z as a single tensor_tensor divide straight from PSUM,
         second horizontal add of the U box, final res = E*S fp16 mul,
         (on 'mix' tiles also the horizontal E adds)
  Pool : first horizontal add of the U box (GpSimd is otherwise idle)
  DMA  : fp16 loads/stores

Row mapping keeps every compute op at partition base 0 (hardware only
allows compute APs to start at partitions 0/32/64/96): the banded
matrices alternate lower/upper diagonals so each stage's output lands
re-centred, and frame-edge partitions hold junk that is either masked
(exp scale), harmless (finite, unused), or skipped by the output DMA
(which may start at any partition).
"""

from contextlib import ExitStack

import numpy as np

import concourse.bacc as bacc
import concourse.mybir as mybir
import concourse.tile as tile
from concourse._compat import with_exitstack
from concourse.bass_utils import run_bass_kernel_spmd
from concourse.dve_ops import RECIP_APPROX_FAST_CONSTS, RECIPROCAL_APPROX_FAST

F16 = mybir.dt.float16
F32 = mybir.dt.float32

H = 4096
W = 4096
N_CORES = 8
RC = H // N_CORES  # rows per core
HALO = 3
RT = 122           # output rows per normal row-tile (RT + 6 <= 128)
C = 512            # matmul column chunk = one fp32 PSUM bank
CG = 1024          # conv PSUM group (2 banks) -> fewer Act instructions
SEGW = 1024        # folded-tile width segment (4 segs on 32-row blocks)

# Which normal tiles compute the Z box via Eh-on-DVE + 1 matmul instead of
# 3 shifted matmuls on the PE (engine balancing knob).
ZMIX = (False, False, False, False)

XW = W + 2 * HALO        # X cols:  j   <-> global col j-3   (4102)
EW = W + 4               # E cols:  e   <-> global col e-2   (4100)
ZW = W + 2               # Z/U cols: c  <-> global col c-1   (4098)


# ---------------------------------------------------------------- host side

def _band(vals, lo):
    """128x128 band matrix: b[p, m] = vals[p-m-lo] for p-m-lo in 0..2."""
    b = np.zeros((128, 128), np.float32)
    idx = np.arange(128)
    for d in range(3):
        off = lo + d
        p = idx[off:] if off >= 0 else idx[: 128 + off]
        m = p - off
        b[p, m] = vals[d]
    return b


def _make_bands(k: np.ndarray) -> np.ndarray:
    """bands[0..2]: conv lhsT per column-shift v (b[p,m]=k[p-m, v]);
    bands[3]: BT ones, lhsT[m,p]=1 for p-m in 0..2 (Z: E-frame -> X-frame);
    bands[4]: BS ones, lhsT[p,m]=1 for p-m in 0..2 (S: X-frame -> E-frame);
    bands[5..9]: the same five as 4x 32x32 block-diagonals (folded tile)."""
    bands = np.zeros((10, 128, 128), np.float32)
    for v in range(3):
        bands[v] = _band(k[:, v], 0)
    bands[3] = _band([1.0, 1.0, 1.0], 0).T
    bands[4] = _band([1.0, 1.0, 1.0], 0)
    for i in range(5):
        for b in range(4):
            s = slice(32 * b, 32 * b + 32)
            bands[5 + i][s, s] = bands[i][:32, :32]
    # p-major [128, 10*128] so the upload is one contiguous 2560B/partition DMA
    return np.ascontiguousarray(bands.transpose(1, 0, 2).reshape(128, -1)).astype(
        np.float16
    )


def _make_core_inputs(x16: np.ndarray, bands: np.ndarray, core: int):
    r0 = core * RC
    # 2 extra rows beyond the +-3 halo let the folded tile load full
    # 32-row blocks (everything the device reads is initialized)
    lo, hi = r0 - HALO, r0 + RC + HALO + 2
    xh = np.zeros((RC + 2 * HALO + 2, XW), np.float16)
    s_lo, s_hi = max(lo, 0), min(hi, H)
    xh[s_lo - lo : s_hi - lo, HALO : HALO + W] = x16[s_lo:s_hi]
    gl = np.arange(r0 - HALO, r0 + RC + HALO)
    mask = ((gl >= 0) & (gl < H)).astype(np.float32)[:, None]
    return {"xh": xh, "mask": mask, "bands": bands}


def _chunks(total: int, step: int):
    out = []
    s = 0
    while s < total:
        out.append((s, min(step, total - s)))
        s += step
    return out


# -------------------------------------------------------------- device side

@with_exitstack
def _energy_body(ctx: ExitStack, tc, out_d, xh_d, mask_d, bands_d):
    nc = tc.nc
    Exp = mybir.ActivationFunctionType.Exp
    Copy = mybir.ActivationFunctionType.Copy
    RC_ = RECIP_APPROX_FAST_CONSTS

    def recip16(out_ap, in_ap):
        # reciprocal_approx_fast with an fp16 output (the DVE output stage
        # downconverts; the fp32 bit-trick only concerns the input)
        nc.vector._custom_dve(
            RECIPROCAL_APPROX_FAST, out=out_ap, in0=in_ap,
            s0=RC_["s0"], s1=RC_["s1"], imm2=RC_["imm2"],
        )

    consts = ctx.enter_context(tc.tile_pool(name="consts", bufs=1))
    bigb = consts.tile([128, 10 * 128], F16, name="bigb")
    nc.sync.dma_start(out=bigb, in_=bands_d)
    Mv = [bigb[:, i * 128 : (i + 1) * 128] for i in range(3)]
    BT = bigb[:, 3 * 128 : 4 * 128]
    BS = bigb[:, 4 * 128 : 5 * 128]
    MvF = [bigb[:, (5 + i) * 128 : (6 + i) * 128] for i in range(3)]
    BTF = bigb[:, 8 * 128 : 9 * 128]
    BSF = bigb[:, 9 * 128 : 10 * 128]

    xpool = ctx.enter_context(tc.tile_pool(name="xp", bufs=3))
    epool = ctx.enter_context(tc.tile_pool(name="ep", bufs=2))
    ehpool = ctx.enter_context(tc.tile_pool(name="ehp", bufs=3))
    upool = ctx.enter_context(tc.tile_pool(name="up", bufs=3))
    uhpool = ctx.enter_context(tc.tile_pool(name="uhp", bufs=2))
    spool = ctx.enter_context(tc.tile_pool(name="sp", bufs=3))
    respool = ctx.enter_context(tc.tile_pool(name="resp", bufs=2))
    mpool = ctx.enter_context(tc.tile_pool(name="mp", bufs=2))
    ps_c = ctx.enter_context(tc.tile_pool(name="psc", bufs=2, space="PSUM"))
    ps_z = ctx.enter_context(tc.tile_pool(name="psz", bufs=2, space="PSUM"))
    ps_s = ctx.enter_context(tc.tile_pool(name="pss", bufs=2, space="PSUM"))

    def normal_early(o, R, zmix):
        """DMA, conv+exp, Z box, reciprocal.  Returns state for the late
        stage; emitted one tile ahead of `normal_late` so each in-order
        engine queue always has next-tile work behind a stalled stage."""
        P = R + 4  # working partitions (E frame); X uses R+6
        mk = mpool.tile([128, 1], F32, tag="mk")
        nc.sync.dma_start(out=mk[:P], in_=mask_d[o + 1 : o + 1 + P, :])

        X = xpool.tile([128, XW], F16, tag="X")
        nc.sync.dma_start(out=X[: R + 6, :], in_=xh_d[o : o + R + 6, :])

        # conv + exp -> E[m, e] <-> (row o-2+m, col e-2); the 4-wide tail
        # group goes first so the right-edge memset clears early
        E = epool.tile([128, EW], F16, tag="E")
        groups = _chunks(EW, CG)
        for g0, gl in groups[-1:] + groups[:-1]:
            pc = ps_c.tile([128, CG], F32, tag="pc")
            for cs, cl in _chunks(gl, C):
                for v in range(3):
                    nc.tensor.matmul(
                        pc[:P, cs : cs + cl],
                        Mv[v][: R + 6, :P],
                        X[: R + 6, g0 + cs + v : g0 + cs + v + cl],
                        start=(v == 0),
                        stop=(v == 2),
                    )
            nc.scalar.activation(E[:P, g0 : g0 + gl], pc[:P, :gl], Exp, scale=mk[:P])
            # pad columns of E represent out-of-grid pixels: exp(0) = 1
            if g0 + gl == EW:
                nc.vector.memset(E[:P, EW - 2 : EW - 1], 1.0)
            if g0 == 0:
                nc.vector.memset(E[:P, 1:2], 1.0)

        if zmix:
            # horizontal E box on DVE, vertical on PE (1 pass)
            eh1 = ehpool.tile([128, ZW], F16, tag="eh1")
            nc.vector.tensor_add(out=eh1[:P, :], in0=E[:P, 0:ZW], in1=E[:P, 1 : ZW + 1])
            eh = ehpool.tile([128, ZW], F16, tag="eh")
            nc.vector.tensor_add(out=eh[:P, :], in0=eh1[:P, :], in1=E[:P, 2 : ZW + 2])

        # Z (X frame via BT), Rz = 1/Z straight from PSUM
        Rz = ehpool.tile([128, ZW], F16, tag="Rz")
        for cs, cl in _chunks(ZW, C):
            pz = ps_z.tile([128, C], F32, tag="pz")
            if zmix:
                nc.tensor.matmul(
                    pz[:P, :cl], BT[:P, :P], eh[:P, cs : cs + cl],
                    start=True, stop=True,
                )
            else:
                for v in range(3):
                    nc.tensor.matmul(
                        pz[:P, :cl], BT[:P, :P], E[:P, cs + v : cs + v + cl],
                        start=(v == 0), stop=(v == 2),
                    )
            recip16(Rz[:P, cs : cs + cl], pz[:P, :cl])
        return o, R, P, X, E, Rz

    def normal_late(state):
        """U = x*Rz, horizontal U box (first add alternates GpSimd/DVE per
        quarter), S vertical (E frame via BS), PSUM -> fp16 via Act copy,
        res = E*S.  Split into column quarters so the chain pipelines at
        ~1us granularity and the PE never starves long enough to drop out
        of its warm p-state."""
        o, R, P, X, E, Rz = state
        U = upool.tile([128, ZW], F16, tag="U")
        uh1 = uhpool.tile([128, W], F16, tag="uh1")
        uh = uhpool.tile([128, W], F16, tag="uh")
        S16 = spool.tile([128, W], F16, tag="S16")
        res = respool.tile([128, W], F16, tag="res")
        QW = W // 4
        for qi in range(4):
            h0 = qi * QW
            # U quarter 0 covers cols [0, QW+2), later ones [h0+2, h0+2+QW)
            u0, ul = (0, QW + 2) if qi == 0 else (h0 + 2, QW)
            nc.vector.tensor_mul(
                out=U[:P, u0 : u0 + ul],
                in0=X[:P, u0 + 2 : u0 + 2 + ul],
                in1=Rz[:P, u0 : u0 + ul],
            )
            eng = nc.gpsimd if qi % 2 == 0 else nc.vector
            eng.tensor_add(
                out=uh1[:P, h0 : h0 + QW],
                in0=U[:P, h0 : h0 + QW],
                in1=U[:P, h0 + 1 : h0 + 1 + QW],
            )
            nc.vector.tensor_add(
                out=uh[:P, h0 : h0 + QW],
                in0=uh1[:P, h0 : h0 + QW],
                in1=U[:P, h0 + 2 : h0 + 2 + QW],
            )
            for cs, cl in _chunks(QW, C):
                ps = ps_s.tile([128, C], F32, tag="ps")
                nc.tensor.matmul(
                    ps[: R + 2, :cl], BS[:P, : R + 2],
                    uh[:P, h0 + cs : h0 + cs + cl],
                    start=True, stop=True,
                )
                nc.scalar.activation(
                    S16[: R + 2, h0 + cs : h0 + cs + cl], ps[: R + 2, :cl], Copy
                )
            nc.vector.tensor_mul(
                out=res[: R + 2, h0 : h0 + QW],
                in0=E[: R + 2, h0 + 2 : h0 + 2 + QW],
                in1=S16[: R + 2, h0 : h0 + QW],
            )
            # valid output rows sit at partitions [2, R+2)
            nc.sync.dma_start(
                out=out_d[o : o + R, h0 : h0 + QW],
                in_=res[2 : R + 2, h0 : h0 + QW],
            )

    def fold_early(o, R):
        # Last 24 rows: 4 width segments of 1024 stacked on 32-partition
        # blocks, block-diagonal bands.  Off-band lanes hold finite junk
        # (masked exp gives E=1 there, X junk rows divide to finite U).
        mk = mpool.tile([128, 1], F32, tag="mk")
        nc.vector.memset(mk, 0.0)
        for b in range(4):
            nc.sync.dma_start(
                out=mk[32 * b : 32 * b + R + 4], in_=mask_d[o + 1 : o + R + 5, :]
            )
        X = xpool.tile([128, SEGW + 6], F16, tag="Xf")
        for b in range(4):
            nc.sync.dma_start(
                out=X[32 * b : 32 * b + 32, :],
                in_=xh_d[o : o + 32, b * SEGW : b * SEGW + SEGW + 6],
            )

        ew, zw = SEGW + 4, SEGW + 2
        E = epool.tile([128, ew], F16, tag="Ef")
        for g0, gl in _chunks(ew, CG):
            pc = ps_c.tile([128, CG], F32, tag="pc")
            for cs, cl in _chunks(gl, C):
                for v in range(3):
                    nc.tensor.matmul(
                        pc[:, cs : cs + cl],
                        MvF[v],
                        X[:, g0 + cs + v : g0 + cs + v + cl],
                        start=(v == 0),
                        stop=(v == 2),
                    )
            nc.scalar.activation(E[:, g0 : g0 + gl], pc[:, :gl], Exp, scale=mk)
        nc.vector.memset(E[0:32, 1:2], 1.0)
        nc.vector.memset(E[96:128, ew - 2 : ew - 1], 1.0)

        Rz = ehpool.tile([128, zw], F16, tag="Rzf")
        for cs, cl in _chunks(zw, C):
            pz = ps_z.tile([128, C], F32, tag="pz")
            for v in range(3):
                nc.tensor.matmul(
                    pz[:, :cl], BTF, E[:, cs + v : cs + v + cl],
                    start=(v == 0), stop=(v == 2),
                )
            recip16(Rz[:, cs : cs + cl], pz[:, :cl])
        return o, R, X, E, Rz

    def fold_late(state):
        o, R, X, E, Rz = state
        zw = SEGW + 2
        U = upool.tile([128, zw], F16, tag="Uf")
        nc.vector.tensor_mul(out=U, in0=X[:, 2 : zw + 2], in1=Rz)

        uh1 = uhpool.tile([128, SEGW], F16, tag="uh1f")
        nc.gpsimd.tensor_add(out=uh1, in0=U[:, 0:SEGW], in1=U[:, 1 : SEGW + 1])
        uh = uhpool.tile([128, SEGW], F16, tag="uhf")
        nc.vector.tensor_add(out=uh, in0=uh1, in1=U[:, 2 : SEGW + 2])

        S16 = spool.tile([128, SEGW], F16, tag="S16f")
        for cs, cl in _chunks(SEGW, C):
            ps = ps_s.tile([128, C], F32, tag="ps")
            nc.tensor.matmul(ps[:, :cl], BSF, uh[:, cs : cs + cl], start=True, stop=True)
            nc.scalar.activation(S16[:, cs : cs + cl], ps[:, :cl], Copy)
        res = respool.tile([128, SEGW], F16, tag="resf")
        nc.vector.tensor_mul(out=res, in0=E[:, 2 : SEGW + 2], in1=S16)
        for b in range(4):
            nc.sync.dma_start(
                out=out_d[o : o + R, b * SEGW : (b + 1) * SEGW],
                in_=res[32 * b + 2 : 32 * b + 2 + R, :],
            )

    with nc.allow_low_precision("fp16 pipeline; verified within tolerance"):
        tiles = _chunks(RC, RT)
        fo, fr = tiles[-1]
        assert fr <= 26
        # 2-stage software pipeline, folded (cheap) tile first: each engine's
        # in-order queue gets tile t+1's early work before tile t's late work
        fold_late(fold_early(fo, fr))
        for i, (o, R) in enumerate(tiles[:-1]):
            normal_late(normal_early(o, R, ZMIX[i % len(ZMIX)]))


_CACHE: dict = {}


def _build():
    if "nc" in _CACHE:
        return _CACHE["nc"]
    nc = bacc.Bacc(
        "TRN2", target_bir_lowering=False, debug=False, num_devices=N_CORES
    )
    xh_d = nc.dram_tensor("xh", (RC + 2 * HALO + 2, XW), F16, kind="ExternalInput").ap()
    mask_d = nc.dram_tensor("mask", (RC + 2 * HALO, 1), F32, kind="ExternalInput").ap()
    bands_d = nc.dram_tensor("bands", (128, 10 * 128), F16, kind="ExternalInput").ap()
    out_d = nc.dram_tensor("out", (RC, W), F16, kind="ExternalOutput").ap()
    with tile.TileContext(nc) as tc:
        _energy_body(tc, out_d, xh_d, mask_d, bands_d)
    nc.compile()
    _CACHE["nc"] = nc
    return nc


def kernel(shareable_energy: np.ndarray, kernel: np.ndarray, **_run_kw) -> np.ndarray:
    x = np.asarray(shareable_energy, np.float32)
    k = np.asarray(kernel, np.float32)
    assert x.shape == (H, W), x.shape
    nc = _build()
    x16 = x.astype(np.float16)
    bands = _make_bands(k)
    in_maps = [_make_core_inputs(x16, bands, core) for core in range(N_CORES)]
    r = run_bass_kernel_spmd(nc, in_maps, core_ids=list(range(N_CORES)), **_run_kw)
    out = np.concatenate(
        [res["out"].astype(np.float32) for res in r.results], axis=0
    )
    if _run_kw:
        _CACHE["last_result"] = r
    return out
